# revision 1
# baseline (speedup 1.0000x reference)
"""Trainium2 Bass kernel for nn_AttrSoftLoss (masked multilabel soft-margin loss).

Reference semantics: per row, drop the k = round(0.95 * n_zero) zero-labeled
positions whose fixed uniform draws (jax.random.key(42)) are smallest, then
average  -[a*log_sigmoid(s) + (1-a)*log_sigmoid(-s)]  over kept positions;
mean over rows.

Key reduction: the uniform matrix is an input-independent constant, so each
row is pre-permuted on the host (constant gather = pure data layout) into
ascending-u order. The dropped set then becomes "the first k zero-labeled
entries of the permuted row", which the device finds with an inclusive
prefix count c of z = (attrs == 0) along the row (native tensor_tensor_scan).
The final sums are permutation-invariant, so nothing is un-permuted.

Per [128, 1024] tile (rows on partitions, permuted classes on free dim):
  z    = 1 - a                 (ScalarE: Copy(a*-1 + 1); exact, a in {0,1})
  zbig = -2000*z               (ScalarE: Copy(a*2000 - 2000))
  sp   = softplus(s) = Ln(Exp(s)*1 + 1)   (ScalarE; one act-table set,
         |s| <= ~5.7 so Exp cannot overflow; table pinned to the single
         set natural_log_exp_and_others holding Copy+Exp+Ln)
  c    = inclusive_prefix_sum(z)          (VectorE tensor_tensor_scan)
  thr  = round_half_even(0.95 * c[:,-1]) - 2000   (magic (x+2^23)-2^23 trick,
         matching jnp.round bit-exactly; all integer arithmetic in f32)
  q    = c + zbig              (GpSimd tensor_tensor; keep <=> q > thr,
         since nonzero-labeled positions get q = c >= 0 > thr always)
  kw  += sum((q > thr) * sp)   (VectorE stt, fused row-accumulate)
  na  += sum((z - 1) * s)      (VectorE stt, = -sum a*s)
using softplus(+-s) = softplus(s) - a*s and m*a = a (only zero-labeled
positions are ever masked):  sum m*softplus(g*s) = kw + na.

loss = (sum kw + sum na) / (B*C), via a [128,1] ones*(1/(B*C)) matmul.
Batch dim B=8192 is sharded 1024 rows per core (pure data parallel); each
core emits its scaled partial scalar and the host unshard sums the 8 floats
(a device AllReduce of 4 bytes costs ~50us + a ~100us+ NEFF entry barrier,
dominating the whole kernel, so the combine is done at gather time).
"""

import numpy as np

B, C = 8192, 1024
N_CORES = 8
ROWS = B // N_CORES  # 1024 rows per core
NB = ROWS // 128     # 8 partition blocks per core
MAGIC = 8388608.0    # 2**23: (x + 2^23) - 2^23 == round-half-even(x)
BIG = 2000.0         # > C, pushes nonzero-labeled positions past any threshold

_cache: dict = {}


def _make_bacc():
    """Plain Bacc. (Pinning the act-table list to a single entry makes the
    emitted act_func_set_id index a different table at runtime — Ln then
    evaluates with the wrong piecewise table — so table selection is left
    stock; Exp/Ln alternating costs some ACT_TABLE_LOADs per block.)"""
    from concourse import bacc

    return bacc.Bacc(
        "TRN2", target_bir_lowering=False, debug=False, num_devices=N_CORES
    )


def _build_nc():
    from concourse import mybir, tile

    Alu = mybir.AluOpType
    Act = mybir.ActivationFunctionType
    f32 = mybir.dt.float32
    i32 = mybir.dt.int32

    nc = _make_bacc()
    scores_d = nc.dram_tensor("scores", [ROWS, C], f32, kind="ExternalInput")
    attrs_d = nc.dram_tensor("attrs", [ROWS, C], i32, kind="ExternalInput")
    out_d = nc.dram_tensor("out", [1, 1], f32, kind="ExternalOutput")

    with tile.TileContext(nc) as tc:
        with (
            tc.tile_pool(name="io", bufs=4) as io,
            tc.tile_pool(name="work", bufs=3) as work,
            tc.tile_pool(name="keep", bufs=1) as keep,
            tc.tile_pool(name="stat", bufs=1) as stat,
            tc.tile_pool(name="psum", bufs=1, space="PSUM") as psum,
        ):
            ones = stat.tile([128, 1], f32)
            nc.vector.memset(ones[:], 1.0 / (B * C))
            # per-block accumulators: col 2b = keep-weighted softplus, 2b+1 = -a*s
            stats = stat.tile([128, 2 * NB], f32)

            # Two phases so the ScalarE runs all Exp ops then all Ln ops:
            # Exp and Ln live in different act-table sets, and alternating
            # them per block reloads a table (~1.3us) twice per block.
            exs, qs, thrs = [], [], []
            for b in range(NB):
                s = io.tile([128, C], f32, tag="s")
                a = io.tile([128, C], i32, tag="a")
                nc.sync.dma_start(out=s[:], in_=scores_d[128 * b : 128 * (b + 1), :])
                nc.sync.dma_start(out=a[:], in_=attrs_d[128 * b : 128 * (b + 1), :])

                z = work.tile([128, C], f32, tag="z")
                nc.scalar.activation(z[:], a[:], Act.Copy, bias=1.0, scale=-1.0)
                ex = keep.tile([128, C], f32, tag=f"ex{b}")
                nc.scalar.activation(ex[:], s[:], Act.Exp)
                exs.append(ex)

                c = work.tile([128, C], f32, tag="c")
                nc.vector.tensor_tensor_scan(
                    c[:], z[:], z[:], 0.0, op0=Alu.add, op1=Alu.bypass
                )
                # thr = rint(0.95 * n_zero) - BIG via the 2^23 magic trick;
                # separate ops so each stage rounds to f32 like XLA does
                t1 = work.tile([128, 1], f32, tag="t1")
                t2 = work.tile([128, 1], f32, tag="t2")
                thr = keep.tile([128, 1], f32, tag=f"thr{b}")
                nc.vector.tensor_scalar(t1[:], c[:, C - 1 : C], 0.95, None, Alu.mult)
                nc.vector.tensor_scalar(t2[:], t1[:], MAGIC, None, Alu.add)
                nc.vector.tensor_scalar(thr[:], t2[:], MAGIC + BIG, None, Alu.subtract)
                thrs.append(thr)

                # q = c - BIG*z fused on VectorE: GpSimd's tensor_tensor SBUF
                # traffic slowed concurrent DVE ops ~30%, a worse trade than
                # one more DVE pass (and it needed an extra ACT pass for -BIG*z)
                q = keep.tile([128, C], f32, tag=f"q{b}")
                nc.vector.scalar_tensor_tensor(
                    q[:], z[:], -BIG, c[:], op0=Alu.mult, op1=Alu.add
                )
                qs.append(q)

                na = work.tile([128, C], f32, tag="na")
                nc.vector.scalar_tensor_tensor(
                    na[:], z[:], 1.0, s[:],
                    op0=Alu.subtract, op1=Alu.mult,
                    accum_out=stats[:, 2 * b + 1 : 2 * b + 2],
                )

            for b in range(NB):
                sp = work.tile([128, C], f32, tag="sp")
                nc.scalar.activation(sp[:], exs[b][:], Act.Ln, bias=1.0)
                kw = work.tile([128, C], f32, tag="kw")
                nc.vector.scalar_tensor_tensor(
                    kw[:], qs[b][:], thrs[b][:], sp[:],
                    op0=Alu.is_gt, op1=Alu.mult,
                    accum_out=stats[:, 2 * b : 2 * b + 1],
                )

            acc = stat.tile([128, 1], f32)
            nc.vector.tensor_reduce(acc[:], stats[:], mybir.AxisListType.X, Alu.add)
            part = psum.tile([1, 1], f32)
            nc.tensor.matmul(part[:], ones[:], acc[:], start=True, stop=True)
            res = stat.tile([1, 1], f32)
            nc.vector.tensor_copy(res[:], part[:])
            nc.sync.dma_start(out=out_d[:, :], in_=res[:])

    nc.compile()
    return nc


def _get_nc():
    if "nc" not in _cache:
        _cache["nc"] = _build_nc()
    return _cache["nc"]


def _get_perm():
    """Constant per-row ascending-argsort of the fixed uniform matrix."""
    if "perm" not in _cache:
        import jax

        with jax.default_device(jax.devices("cpu")[0]):
            u = np.asarray(jax.random.uniform(jax.random.key(42), (B, C)))
        _cache["perm"] = np.argsort(u, axis=1, kind="stable")
    return _cache["perm"]


def _make_in_maps(scores: np.ndarray, attributes: np.ndarray):
    perm = _get_perm()
    s_p = np.take_along_axis(np.asarray(scores, dtype=np.float32), perm, axis=1)
    a_p = np.take_along_axis(np.asarray(attributes, dtype=np.int32), perm, axis=1)
    in_maps = []
    for i in range(N_CORES):
        r0, r1 = i * ROWS, (i + 1) * ROWS
        in_maps.append(
            {
                "scores": np.ascontiguousarray(s_p[r0:r1]),
                "attrs": np.ascontiguousarray(a_p[r0:r1]),
            }
        )
    return in_maps


def _run(in_maps, trace=False, **kwargs):
    from concourse import bass_utils

    return bass_utils.run_bass_kernel_spmd(
        _get_nc(), in_maps, core_ids=list(range(N_CORES)), trace=trace, **kwargs
    )


def kernel(scores: np.ndarray, attributes: np.ndarray) -> np.ndarray:
    res = _run(_make_in_maps(scores, attributes))
    parts = np.stack(
        [np.asarray(r["out"], dtype=np.float32).reshape(()) for r in res.results]
    )
    return np.float32(np.sum(parts, dtype=np.float32)).reshape(())[()]



# revision 4
# speedup vs baseline: 1.5365x; 1.5365x over previous
"""Trainium2 Bass kernel for nn_AttrSoftLoss (masked multilabel soft-margin loss).

Reference semantics: per row, drop the k = round(0.95 * n_zero) zero-labeled
positions whose fixed uniform draws (jax.random.key(42)) are smallest, then
average  -[a*log_sigmoid(s) + (1-a)*log_sigmoid(-s)]  over kept positions;
mean over rows.  loss = [sum_kept softplus(s) - sum_all a*s] / (B*C).

Layout: the uniform matrix is an input-independent constant, so each row is
pre-permuted on the host into ascending-u order (pure data layout); the
dropped set becomes "the first k zero-labeled entries" in storage order.
Data is stored TRANSPOSED (classes on partitions, rows on the free dim) so
the per-row inclusive prefix count of zeros becomes a lower-triangular
matmul on the otherwise-idle PE array, replacing the DVE tensor_tensor_scan
(2.7us/tile, no fast mode) of the previous version.

Mask without a per-row threshold barrier: kept <=> c > rint(0.95*nz), with
c = prefix-zero-count, nz = row zero count.  Using the integer-exact scaled
form Q = 20c + 20*KK*a - 19*nz (attrs are host-encoded as A = 20a in fp16 so
19*nz = 19456 - 0.95*sum(A) stays integer-exact: 0.95*(20k) = 19k), kept
<=> Q > 10.4.  Everything row-dependent is linear in A, so per [128,1024]
class-block cb:

    q_psum = (KK*I - U)@A_cb + J@F_cb          (4 matmuls, f32-exact ints)
    F_cb   = 0.95*A_total - A_prev_cb          (fp16 chain, integer-exact)
    kept  <=> q_psum > thr_cb[i]               (host f32 per-partition const)

and the whole mask+multiply+reduce is ONE fused DVE op per tile:
    stt(scr, q_psum, thr_ptr, sp, is_gt, mult, accum_out=stats).
The only deviation from the reference is the round-half-even tie on ~2.5%
of rows (+ the designed 0.52 cut): ~234 of 8.4M mask elements differ,
rel err ~5e-5 (numpy-verified), far under the 2e-2 gate.

ScalarE computes softplus(s) = Ln(1 + Exp(s)) in fp16; DVE additionally runs
the A_total/F chains (fp16 4x mode) and sum(A*s) (tt + ts-accum); final
per-core scalar = [sum kept_sp]/(B*C) - [sum A*s]/(20*B*C) via two tiny
f32 matmuls into one PSUM accumulator.  Batch is sharded 1024 rows/core
(pure data parallel); the host sums the 8 partial scalars at gather time
(a 4-byte device AllReduce costs ~50us+barrier, dominating the kernel).
"""

import numpy as np

B, C = 8192, 1024
N_CORES = 8
ROWS = B // N_CORES  # 1024 rows per core (free dim after transpose)
NB = C // 128        # 8 class-blocks per core (partition dim)
KK = 1024.0          # ones-pusher: 20*KK > 19*1024 + cut
CUT = 10.4           # 20-scaled keep cut: integer Q kept iff Q > 10.4

_cache: dict = {}


def _build_nc():
    from concourse import bacc, mybir, tile

    Alu = mybir.AluOpType
    Act = mybir.ActivationFunctionType
    f32 = mybir.dt.float32
    f16 = mybir.dt.float16

    nc = bacc.Bacc(
        "TRN2", target_bir_lowering=False, debug=False, num_devices=N_CORES
    )
    s_d = nc.dram_tensor("s", [C, ROWS], f16, kind="ExternalInput")
    a_d = nc.dram_tensor("a20", [C, ROWS], f16, kind="ExternalInput")
    w_d = nc.dram_tensor("wtri", [128, 128], f16, kind="ExternalInput")
    thr_d = nc.dram_tensor("thr", [128, NB], f32, kind="ExternalInput")
    out_d = nc.dram_tensor("out", [1, 1], f32, kind="ExternalOutput")

    with tile.TileContext(nc) as tc:
        with (
            tc.tile_pool(name="io", bufs=1) as io,
            tc.tile_pool(name="chain", bufs=2) as chain,
            tc.tile_pool(name="keep", bufs=1) as keep,
            tc.tile_pool(name="work", bufs=3) as work,
            tc.tile_pool(name="stat", bufs=1) as stat,
            tc.tile_pool(name="psum", bufs=3, space="PSUM") as psum,
            tc.tile_pool(name="psum_out", bufs=1, space="PSUM") as psum_out,
        ):
            wtri = stat.tile([128, 128], f16)
            thr = stat.tile([128, NB], f32)
            jmat = stat.tile([128, 128], f16)
            stats = stat.tile([128, 2 * NB], f32)
            nc.sync.dma_start(out=wtri[:], in_=w_d[:, :])
            nc.sync.dma_start(out=thr[:], in_=thr_d[:, :])
            nc.vector.memset(jmat[:], 1.0)

            # Load all A blocks first (the F chain gates the PE phase), then s.
            A = []
            S = []
            for cb in range(NB):
                a = io.tile([128, ROWS], f16, tag=f"A{cb}")
                nc.sync.dma_start(out=a[:], in_=a_d[128 * cb : 128 * (cb + 1), :])
                A.append(a)
            for cb in range(NB):
                s = io.tile([128, ROWS], f16, tag=f"S{cb}")
                nc.sync.dma_start(out=s[:], in_=s_d[128 * cb : 128 * (cb + 1), :])
                S.append(s)

            # A_total chain (fp16 integers <= 160, exact; DVE 4x mode).
            at = A[0]
            for cb in range(1, NB):
                nxt = chain.tile([128, ROWS], f16, tag="at")
                nc.vector.tensor_tensor(nxt[:], at[:], A[cb][:], Alu.add)
                at = nxt
            # F chain: F_0 = 0.95*A_total (= exact 19*a_count ints in fp16),
            # F_{cb+1} = F_cb - A_cb.
            F = [None] * NB
            f0 = keep.tile([128, ROWS], f16, tag="F0")
            nc.vector.tensor_scalar(f0[:], at[:], 0.95, None, Alu.mult)
            F[0] = f0
            for cb in range(1, NB):
                nxt = keep.tile([128, ROWS], f16, tag=f"F{cb}")
                nc.vector.tensor_tensor(nxt[:], F[cb - 1][:], A[cb - 1][:], Alu.subtract)
                F[cb] = nxt

            # softplus(s) = Ln(1 + Exp(s)); per-tile interleaved (Exp and Ln
            # ideally share the natural_log_exp_and_others act table).
            ex = [None] * NB
            for cb in range(NB):
                e = keep.tile([128, ROWS], f16, tag=f"ex{cb}")
                nc.scalar.activation(e[:], S[cb][:], Act.Exp)
                ex[cb] = e

            # sum(A*s) per tile: tt product + ts-accumulate (both 4x).
            for cb in range(NB):
                as_ = work.tile([128, ROWS], f16, tag="as")
                nc.vector.tensor_tensor(as_[:], A[cb][:], S[cb][:], Alu.mult)
                scr_a = work.tile([128, ROWS], f16, tag="scra")
                nc.vector.tensor_scalar(
                    scr_a[:], as_[:], 1.0, 0.0, Alu.mult, Alu.add,
                    accum_out=stats[:, NB + cb : NB + cb + 1],
                )

            for cb in range(NB):
                sp = work.tile([128, ROWS], f16, tag="sp")
                nc.scalar.activation(sp[:], ex[cb][:], Act.Ln, bias=1.0)

                q = psum.tile([128, ROWS], f32, tag="q")
                for h in range(2):
                    sl = slice(512 * h, 512 * (h + 1))
                    nc.tensor.matmul(
                        q[:, sl], wtri[:], A[cb][:, sl], start=True, stop=False
                    )
                    nc.tensor.matmul(
                        q[:, sl], jmat[:], F[cb][:, sl], start=False, stop=True
                    )
                scr = work.tile([128, ROWS], f16, tag="scr")
                nc.vector.scalar_tensor_tensor(
                    scr[:], q[:], thr[:, cb : cb + 1], sp[:],
                    op0=Alu.is_gt, op1=Alu.mult,
                    accum_out=stats[:, cb : cb + 1],
                )

            acc1 = stat.tile([128, 1], f32)
            acc2 = stat.tile([128, 1], f32)
            nc.vector.tensor_reduce(
                acc1[:], stats[:, 0:NB], mybir.AxisListType.X, Alu.add
            )
            nc.vector.tensor_reduce(
                acc2[:], stats[:, NB : 2 * NB], mybir.AxisListType.X, Alu.add
            )
            ones_a = stat.tile([128, 1], f32)
            ones_b = stat.tile([128, 1], f32)
            nc.vector.memset(ones_a[:], 1.0 / (B * C))
            nc.vector.memset(ones_b[:], -1.0 / (20.0 * B * C))
            part = psum_out.tile([1, 1], f32)
            nc.tensor.matmul(part[:], ones_a[:], acc1[:], start=True, stop=False)
            nc.tensor.matmul(part[:], ones_b[:], acc2[:], start=False, stop=True)
            res = stat.tile([1, 1], f32)
            nc.vector.tensor_copy(res[:], part[:])
            nc.sync.dma_start(out=out_d[:, :], in_=res[:])

    nc.compile()
    return nc


def _get_nc():
    if "nc" not in _cache:
        _cache["nc"] = _build_nc()
    return _cache["nc"]


def _get_perm():
    """Constant per-row ascending-argsort of the fixed uniform matrix."""
    if "perm" not in _cache:
        import jax

        with jax.default_device(jax.devices("cpu")[0]):
            u = np.asarray(jax.random.uniform(jax.random.key(42), (B, C)))
        _cache["perm"] = np.argsort(u, axis=1, kind="stable")
    return _cache["perm"]


def _consts():
    if "wtri" not in _cache:
        # lhsT[k,i] = KK*delta_ki - [k <= i]  (matmul computes lhsT.T @ rhs)
        w = -np.triu(np.ones((128, 128), np.float32)) + KK * np.eye(128)
        _cache["wtri"] = w.astype(np.float16)
        i = np.arange(128, dtype=np.float64)[:, None]
        cb = np.arange(NB, dtype=np.float64)[None, :]
        thr = 19456.0 + CUT - 2560.0 * cb - 20.0 * (i + 1.0)
        _cache["thr"] = thr.astype(np.float32)
    return _cache["wtri"], _cache["thr"]


def _make_in_maps(scores: np.ndarray, attributes: np.ndarray):
    perm = _get_perm()
    s_p = np.take_along_axis(np.asarray(scores, dtype=np.float32), perm, axis=1)
    a_p = np.take_along_axis(np.asarray(attributes, dtype=np.int32), perm, axis=1)
    s16 = s_p.astype(np.float16)
    a20 = (20 * a_p).astype(np.float16)
    wtri, thr = _consts()
    in_maps = []
    for i in range(N_CORES):
        r0, r1 = i * ROWS, (i + 1) * ROWS
        in_maps.append(
            {
                "s": np.ascontiguousarray(s16[r0:r1].T),
                "a20": np.ascontiguousarray(a20[r0:r1].T),
                "wtri": wtri,
                "thr": thr,
            }
        )
    return in_maps


def _run(in_maps, trace=False, **kwargs):
    from concourse import bass_utils

    return bass_utils.run_bass_kernel_spmd(
        _get_nc(), in_maps, core_ids=list(range(N_CORES)), trace=trace, **kwargs
    )


def kernel(scores: np.ndarray, attributes: np.ndarray) -> np.ndarray:
    res = _run(_make_in_maps(scores, attributes))
    parts = np.stack(
        [np.asarray(r["out"], dtype=np.float32).reshape(()) for r in res.results]
    )
    return np.float32(np.sum(parts, dtype=np.float32)).reshape(())[()]
